# revision 7
# baseline (speedup 1.0000x reference)
"""GroupedEmbedding lookup on 8 Trainium2 NeuronCores.

Problem: 8 tables [100000, 128] f32, 8 index vectors [200000] int64.
Output: per-table gather concatenated -> [1600000, 128] f32.

Sharding: table-parallel. Core c holds table c and its 200000 indices;
it gathers locally. No collectives. Host concatenates the 8 slices.

Per-core kernel (v3, bucket-bipartite with Q7 ant ucode):
  indirect1d DMA is capped at 128 rows (1 index/partition) per ~1us
  SWDGE instruction -> 1568 instr = 1.7ms engine-serialized (the old
  baseline). Instead use the mlp-library Q7 ops:
    dma_gather      (<=1024 int16 idxs/instr, table chunk <=32768 rows)
    dma_scatter_add (<=3072 int16 idxs/instr, window <=32768 rows, +=)
  Host sorts each core's rows into 28 buckets = (7 output windows) x
  (4 table chunks of 25000 rows), padded to static caps -> the program
  is input-independent. Each window spans 32768 out-tensor rows but
  holds only 32256 real rows; pad scatters target in-window row 32767
  (reserved garbage) so their += can never race a real row's value.
  The PJRT run path zero-donates output buffers, so += = assignment.
  Work is pipelined in uniform units of 3072 rows (3 gathers + 1
  scatter) over NBUF slot buffers -> ~300 SWDGE instructions total
  instead of 1568, with gather and scatter DMAs overlapping.
"""

import os
import sys

for _p in ("/root/.axon_site", "/root/.axon_site/_ro/trn_rl_repo",
           "/root/.axon_site/_ro/pypackages", "/opt/trn_rl_repo"):
    if os.path.isdir(_p) and _p not in sys.path:
        sys.path.append(_p)

from contextlib import ExitStack

import numpy as np

import concourse.bacc as bacc
import concourse.mybir as mybir
from concourse.bass_utils import run_bass_kernel_spmd
from concourse.library_config import mlp

NUM_TABLES = 8
NUM_EMBEDDINGS = 100000
EMBED_DIM = 128
IDS_PER_FEATURE = 200000

P = 128
ROWS_PAD = 200704            # 1568 * 128 (704 pad rows, idx 0)
WSPAN = 32768                # out-tensor rows per window
WREAL = 32256                # real rows per window; rest is garbage pad
GARBAGE = WSPAN - 1          # in-window scatter target for pad slots
NWIN = 7                     # 7 * 32256 = 225792 >= 200704
OUT_ROWS = NWIN * WSPAN      # 229376
CH = 25000                   # table chunk rows (int16 gather range)
NCHUNK = 4
GCAP = 1024                  # rows per dma_gather instruction
UNIT = 3072                  # rows per unit = 3 gathers + 1 scatter
# static bucket caps (rows): full windows Bin(32256,1/4)=8064+-78 ->
# 9216 (+15sig); last window (7168 rows incl 704 idx-0 pads in chunk 0:
# ~2320 max expected) -> 3072.
CAP_FULL = 9216
CAP_LAST = 3072
NBUF = 6                     # slot-buffer pipeline depth

BUCKETS = []                 # (window, chunk, cap, idx_col_offset)
_off = 0
for _w in range(NWIN):
    cap = CAP_FULL if _w < NWIN - 1 else CAP_LAST
    for _c in range(NCHUNK):
        BUCKETS.append((_w, _c, cap, _off))
        _off += cap // 16
IDX_COLS = _off
# uniform pipeline units of UNIT rows
UNITS = []                   # (window, chunk, idx_col_offset_of_unit)
for _w, _c, _cap, _boff in BUCKETS:
    for _k in range(_cap // UNIT):
        UNITS.append((_w, _c, _boff + _k * (UNIT // 16)))


def build_nc():
    """Per-core Bass program (SPMD: same static program on all cores)."""
    nc = bacc.Bacc("TRN2")
    gidx = nc.dram_tensor("gidx", [P, IDX_COLS], mybir.dt.int16,
                          kind="ExternalInput")
    sidx = nc.dram_tensor("sidx", [P, IDX_COLS], mybir.dt.int16,
                          kind="ExternalInput")
    tab = nc.dram_tensor("tab", [NUM_EMBEDDINGS, EMBED_DIM],
                         mybir.dt.float32, kind="ExternalInput")
    out = nc.dram_tensor("out", [OUT_ROWS, EMBED_DIM], mybir.dt.float32,
                         kind="ExternalOutput")

    nu = len(UNITS)
    with ExitStack() as es:
        block = es.enter_context(nc.Block())
        i_sem = es.enter_context(nc.semaphore("i_sem"))
        g_sems = [es.enter_context(nc.semaphore(f"g_sem{r}"))
                  for r in range(NBUF)]
        s_sems = [es.enter_context(nc.semaphore(f"s_sem{r}"))
                  for r in range(NBUF)]
        gidx_sb = es.enter_context(
            nc.sbuf_tensor("gidx_sb", [P, IDX_COLS], mybir.dt.int16))
        sidx_sb = es.enter_context(
            nc.sbuf_tensor("sidx_sb", [P, IDX_COLS], mybir.dt.int16))
        slots = [
            es.enter_context(
                nc.sbuf_tensor(f"slots{r}", [P, UNIT // P, EMBED_DIM],
                               mybir.dt.float32))
            for r in range(NBUF)
        ]

        g_done = [0] * NBUF   # gathers issued per buffer (build-time count)
        s_done = [0] * NBUF

        def gathers(gp, u):
            w, c, off = UNITS[u]
            r = u % NBUF
            for k in range(UNIT // GCAP):
                gp.dma_gather(
                    out_ap=slots[r][:, k * (GCAP // P):(k + 1) * (GCAP // P), :],
                    in_ap=tab[c * CH:(c + 1) * CH, :],
                    idxs_ap=gidx_sb[:, off + k * (GCAP // 16):
                                    off + (k + 1) * (GCAP // 16)],
                    num_idxs=GCAP,
                    num_idxs_reg=GCAP,
                    elem_size=EMBED_DIM,
                ).then_inc(g_sems[r], 16)
            g_done[r] += UNIT // GCAP

        def scatter(gp, u):
            w, c, off = UNITS[u]
            r = u % NBUF
            gp.dma_scatter_add(
                out_ap=out[w * WSPAN:(w + 1) * WSPAN, :],
                in_ap=slots[r][:, :, :],
                idxs_ap=sidx_sb[:, off:off + UNIT // 16],
                num_idxs=UNIT,
                num_idxs_reg=UNIT,
                elem_size=EMBED_DIM,
            ).then_inc(s_sems[r], 16)
            s_done[r] += 1

        @block.gpsimd
        def _(gp):
            gp.load_library(mlp)
            gp.dma_start(out=gidx_sb[:, :], in_=gidx[:, :]).then_inc(i_sem, 16)
            gp.dma_start(out=sidx_sb[:, :], in_=sidx[:, :]).then_inc(i_sem, 16)
            gp.wait_ge(i_sem, 32)
            for u in range(min(NBUF, nu)):
                gathers(gp, u)
            for u in range(nu):
                r = u % NBUF
                gp.wait_ge(g_sems[r], 16 * g_done[r])
                scatter(gp, u)
                if u + NBUF < nu:
                    gp.wait_ge(s_sems[r], 16 * s_done[r])
                    gathers(gp, u + NBUF)
            for r in range(NBUF):
                if s_done[r]:
                    gp.wait_ge(s_sems[r], 16 * s_done[r])
    nc.compile()
    return nc


_NC_CACHE = {}


def _get_nc():
    if "nc" not in _NC_CACHE:
        _NC_CACHE["nc"] = build_nc()
    return _NC_CACHE["nc"]


def _wrap16(v):
    """[N] int16 -> [128, N/16] wrapped in 16 partitions, replicated x8."""
    w = v.reshape(-1, 16).T
    return np.ascontiguousarray(np.tile(w, (8, 1)))


def _build_core_idx(idx_i32):
    """Bucket-sort one core's padded indices into gidx/sidx tensors."""
    gi_all = np.empty(IDX_COLS * 16, dtype=np.int16)
    si_all = np.empty(IDX_COLS * 16, dtype=np.int16)
    pos = np.arange(ROWS_PAD, dtype=np.int32)
    w_of = pos // WREAL
    r_of = pos % WREAL
    chunk_of = idx_i32 // CH
    for w, c, cap, off in BUCKETS:
        sel = pos[(w_of == w) & (chunk_of == c)]
        n = len(sel)
        assert n <= cap, f"bucket ({w},{c}) overflow: {n} > {cap}"
        g = np.zeros(cap, dtype=np.int16)            # pad -> chunk row 0
        s = np.full(cap, GARBAGE, dtype=np.int16)    # pad -> garbage row
        g[:n] = (idx_i32[sel] - c * CH).astype(np.int16)
        s[:n] = r_of[sel].astype(np.int16)
        o16 = off * 16
        gi_all[o16:o16 + cap] = g
        si_all[o16:o16 + cap] = s
    return {"gidx": _wrap16(gi_all), "sidx": _wrap16(si_all)}


def run(values: np.ndarray, weights: np.ndarray, trace: bool = False, **kw):
    assert values.shape == (NUM_TABLES, IDS_PER_FEATURE)
    assert weights.shape == (NUM_TABLES, NUM_EMBEDDINGS, EMBED_DIM)

    nc = _get_nc()

    idx_pad = np.zeros((NUM_TABLES, ROWS_PAD), dtype=np.int32)
    idx_pad[:, :IDS_PER_FEATURE] = values.astype(np.int32)
    w = np.ascontiguousarray(weights, dtype=np.float32)
    in_maps = [
        {**_build_core_idx(idx_pad[c]), "tab": w[c]}
        for c in range(NUM_TABLES)
    ]
    res = run_bass_kernel_spmd(nc, in_maps, core_ids=list(range(NUM_TABLES)),
                               trace=trace, **kw)
    outs = [
        r["out"].reshape(NWIN, WSPAN, EMBED_DIM)[:, :WREAL]
        .reshape(-1, EMBED_DIM)[:IDS_PER_FEATURE]
        for r in res.results
    ]
    return np.concatenate(outs, axis=0), res


def kernel(values: np.ndarray, weights: np.ndarray) -> np.ndarray:
    return run(values, weights)[0]


# revision 8
# speedup vs baseline: 1.0491x; 1.0491x over previous
"""GroupedEmbedding lookup on 8 Trainium2 NeuronCores.

Problem: 8 tables [100000, 128] f32, 8 index vectors [200000] int64.
Output: per-table gather concatenated -> [1600000, 128] f32.

Sharding: table-parallel. Core c holds table c and its 200000 indices;
it gathers locally. No collectives. Host concatenates the 8 slices.

Per-core kernel (v3, bucket-bipartite with Q7 ant ucode):
  indirect1d DMA is capped at 128 rows (1 index/partition) per ~1us
  SWDGE instruction -> 1568 instr = 1.7ms engine-serialized (the old
  baseline). Instead use the mlp-library Q7 ops:
    dma_gather      (<=1024 int16 idxs/instr, table chunk <=32768 rows)
    dma_scatter_add (<=3072 int16 idxs/instr, window <=32768 rows, +=)
  Host sorts each core's rows into 28 buckets = (7 output windows) x
  (4 table chunks of 25000 rows), padded to static caps -> the program
  is input-independent. Each window spans 32768 out-tensor rows but
  holds only 32256 real rows; pad scatters target in-window row 32767
  (reserved garbage) so their += can never race a real row's value.
  The PJRT run path zero-donates output buffers, so += = assignment.
  Work is pipelined in uniform units of 3072 rows (3 gathers + 1
  scatter) over NBUF slot buffers -> ~300 SWDGE instructions total
  instead of 1568, with gather and scatter DMAs overlapping.
"""

import os
import sys

for _p in ("/root/.axon_site", "/root/.axon_site/_ro/trn_rl_repo",
           "/root/.axon_site/_ro/pypackages", "/opt/trn_rl_repo"):
    if os.path.isdir(_p) and _p not in sys.path:
        sys.path.append(_p)

from contextlib import ExitStack

import numpy as np

import concourse.bacc as bacc
import concourse.mybir as mybir
from concourse.bass_utils import run_bass_kernel_spmd
from concourse.library_config import mlp

NUM_TABLES = 8
NUM_EMBEDDINGS = 100000
EMBED_DIM = 128
IDS_PER_FEATURE = 200000

P = 128
ROWS_PAD = 200704            # 1568 * 128 (704 pad rows, idx 0)
WSPAN = 32768                # out-tensor rows per window
WREAL = 32256                # real rows per window; rest is garbage pad
GARBAGE = WSPAN - 1          # in-window scatter target for pad slots
NWIN = 7                     # 7 * 32256 = 225792 >= 200704
OUT_ROWS = NWIN * WSPAN      # 229376
CH = 25000                   # table chunk rows (int16 gather range)
NCHUNK = 4
GCAP = 1024                  # rows per dma_gather instruction
UNIT = 3072                  # rows per unit = 3 gathers + 1 scatter
# static bucket caps (rows): full windows Bin(32256,1/4)=8064+-78 ->
# 9216 (+15sig); last window (7168 rows incl 704 idx-0 pads in chunk 0:
# ~2320 max expected) -> 3072.
CAP_FULL = 9216
CAP_LAST = 3072
NBUF = 6                     # slot-buffer pipeline depth

BUCKETS = []                 # (window, chunk, cap, idx_col_offset)
_off = 0
for _w in range(NWIN):
    cap = CAP_FULL if _w < NWIN - 1 else CAP_LAST
    for _c in range(NCHUNK):
        BUCKETS.append((_w, _c, cap, _off))
        _off += cap // 16
IDX_COLS = _off
# uniform pipeline units of UNIT rows
UNITS = []                   # (window, chunk, idx_col_offset_of_unit)
for _w, _c, _cap, _boff in BUCKETS:
    for _k in range(_cap // UNIT):
        UNITS.append((_w, _c, _boff + _k * (UNIT // 16)))


def build_nc():
    """Per-core Bass program (SPMD: same static program on all cores)."""
    nc = bacc.Bacc("TRN2")
    gidx = nc.dram_tensor("gidx", [P, IDX_COLS], mybir.dt.int16,
                          kind="ExternalInput")
    sidx = nc.dram_tensor("sidx", [P, IDX_COLS], mybir.dt.int16,
                          kind="ExternalInput")
    tab = nc.dram_tensor("tab", [NUM_EMBEDDINGS, EMBED_DIM],
                         mybir.dt.float32, kind="ExternalInput")
    out = nc.dram_tensor("out", [OUT_ROWS, EMBED_DIM], mybir.dt.float32,
                         kind="ExternalOutput")

    nu = len(UNITS)
    with ExitStack() as es:
        block = es.enter_context(nc.Block())
        i_sem = es.enter_context(nc.semaphore("i_sem"))
        g_sems = [es.enter_context(nc.semaphore(f"g_sem{r}"))
                  for r in range(NBUF)]
        s_sems = [es.enter_context(nc.semaphore(f"s_sem{r}"))
                  for r in range(NBUF)]
        gidx_sb = es.enter_context(
            nc.sbuf_tensor("gidx_sb", [P, IDX_COLS], mybir.dt.int16))
        sidx_sb = es.enter_context(
            nc.sbuf_tensor("sidx_sb", [P, IDX_COLS], mybir.dt.int16))
        slots = [
            es.enter_context(
                nc.sbuf_tensor(f"slots{r}", [P, UNIT // P, EMBED_DIM],
                               mybir.dt.float32))
            for r in range(NBUF)
        ]

        g_done = [0] * NBUF   # gathers issued per buffer (build-time count)
        s_done = [0] * NBUF

        def gathers(gp, u):
            w, c, off = UNITS[u]
            r = u % NBUF
            for k in range(UNIT // GCAP):
                gp.dma_gather(
                    out_ap=slots[r][:, k * (GCAP // P):(k + 1) * (GCAP // P), :],
                    in_ap=tab[c * CH:(c + 1) * CH, :],
                    idxs_ap=gidx_sb[:, off + k * (GCAP // 16):
                                    off + (k + 1) * (GCAP // 16)],
                    num_idxs=GCAP,
                    num_idxs_reg=GCAP,
                    elem_size=EMBED_DIM,
                    single_packet=False,
                ).then_inc(g_sems[r], 16)
            g_done[r] += UNIT // GCAP

        def scatter(gp, u):
            w, c, off = UNITS[u]
            r = u % NBUF
            gp.dma_scatter_add(
                out_ap=out[w * WSPAN:(w + 1) * WSPAN, :],
                in_ap=slots[r][:, :, :],
                idxs_ap=sidx_sb[:, off:off + UNIT // 16],
                num_idxs=UNIT,
                num_idxs_reg=UNIT,
                elem_size=EMBED_DIM,
                single_packet=False,
            ).then_inc(s_sems[r], 16)
            s_done[r] += 1

        @block.gpsimd
        def _(gp):
            gp.load_library(mlp)
            gp.dma_start(out=gidx_sb[:, :], in_=gidx[:, :]).then_inc(i_sem, 16)
            gp.dma_start(out=sidx_sb[:, :], in_=sidx[:, :]).then_inc(i_sem, 16)
            gp.wait_ge(i_sem, 32)
            for u in range(min(NBUF, nu)):
                gathers(gp, u)
            for u in range(nu):
                r = u % NBUF
                gp.wait_ge(g_sems[r], 16 * g_done[r])
                scatter(gp, u)
                if u + NBUF < nu:
                    gp.wait_ge(s_sems[r], 16 * s_done[r])
                    gathers(gp, u + NBUF)
            for r in range(NBUF):
                if s_done[r]:
                    gp.wait_ge(s_sems[r], 16 * s_done[r])
    nc.compile()
    return nc


_NC_CACHE = {}


def _get_nc():
    if "nc" not in _NC_CACHE:
        _NC_CACHE["nc"] = build_nc()
    return _NC_CACHE["nc"]


def _wrap16(v):
    """[N] int16 -> [128, N/16] wrapped in 16 partitions, replicated x8."""
    w = v.reshape(-1, 16).T
    return np.ascontiguousarray(np.tile(w, (8, 1)))


def _build_core_idx(idx_i32):
    """Bucket-sort one core's padded indices into gidx/sidx tensors."""
    gi_all = np.empty(IDX_COLS * 16, dtype=np.int16)
    si_all = np.empty(IDX_COLS * 16, dtype=np.int16)
    pos = np.arange(ROWS_PAD, dtype=np.int32)
    w_of = pos // WREAL
    r_of = pos % WREAL
    chunk_of = idx_i32 // CH
    for w, c, cap, off in BUCKETS:
        sel = pos[(w_of == w) & (chunk_of == c)]
        n = len(sel)
        assert n <= cap, f"bucket ({w},{c}) overflow: {n} > {cap}"
        g = np.zeros(cap, dtype=np.int16)            # pad -> chunk row 0
        s = np.full(cap, GARBAGE, dtype=np.int16)    # pad -> garbage row
        g[:n] = (idx_i32[sel] - c * CH).astype(np.int16)
        s[:n] = r_of[sel].astype(np.int16)
        o16 = off * 16
        gi_all[o16:o16 + cap] = g
        si_all[o16:o16 + cap] = s
    return {"gidx": _wrap16(gi_all), "sidx": _wrap16(si_all)}


def run(values: np.ndarray, weights: np.ndarray, trace: bool = False, **kw):
    assert values.shape == (NUM_TABLES, IDS_PER_FEATURE)
    assert weights.shape == (NUM_TABLES, NUM_EMBEDDINGS, EMBED_DIM)

    nc = _get_nc()

    idx_pad = np.zeros((NUM_TABLES, ROWS_PAD), dtype=np.int32)
    idx_pad[:, :IDS_PER_FEATURE] = values.astype(np.int32)
    w = np.ascontiguousarray(weights, dtype=np.float32)
    in_maps = [
        {**_build_core_idx(idx_pad[c]), "tab": w[c]}
        for c in range(NUM_TABLES)
    ]
    res = run_bass_kernel_spmd(nc, in_maps, core_ids=list(range(NUM_TABLES)),
                               trace=trace, **kw)
    outs = [
        r["out"].reshape(NWIN, WSPAN, EMBED_DIM)[:, :WREAL]
        .reshape(-1, EMBED_DIM)[:IDS_PER_FEATURE]
        for r in res.results
    ]
    return np.concatenate(outs, axis=0), res


def kernel(values: np.ndarray, weights: np.ndarray) -> np.ndarray:
    return run(values, weights)[0]


# revision 9
# speedup vs baseline: 1.1667x; 1.1122x over previous
"""GroupedEmbedding lookup on 8 Trainium2 NeuronCores.

Problem: 8 tables [100000, 128] f32, 8 index vectors [200000] int64.
Output: per-table gather concatenated -> [1600000, 128] f32.

Sharding: table-parallel. Core c holds table c and its 200000 indices;
it gathers locally. No collectives. Host concatenates the 8 slices.

Per-core kernel (v3, bucket-bipartite with Q7 ant ucode):
  indirect1d DMA is capped at 128 rows (1 index/partition) per ~1us
  SWDGE instruction -> 1568 instr = 1.7ms engine-serialized (the old
  baseline). Instead use the mlp-library Q7 ops:
    dma_gather      (<=1024 int16 idxs/instr, table chunk <=32768 rows)
    dma_scatter_add (<=3072 int16 idxs/instr, window <=32768 rows, +=)
  Host sorts each core's rows into 28 buckets = (7 output windows) x
  (4 table chunks of 25000 rows), padded to static caps -> the program
  is input-independent. Each window spans 32768 out-tensor rows but
  holds only 32256 real rows; pad scatters target in-window row 32767
  (reserved garbage) so their += can never race a real row's value.
  The PJRT run path zero-donates output buffers, so += = assignment.
  Work is pipelined in uniform units of 3072 rows (3 gathers + 1
  scatter) over NBUF slot buffers -> ~300 SWDGE instructions total
  instead of 1568, with gather and scatter DMAs overlapping.
"""

import os
import sys

for _p in ("/root/.axon_site", "/root/.axon_site/_ro/trn_rl_repo",
           "/root/.axon_site/_ro/pypackages", "/opt/trn_rl_repo"):
    if os.path.isdir(_p) and _p not in sys.path:
        sys.path.append(_p)

from contextlib import ExitStack

import numpy as np

import concourse.bacc as bacc
import concourse.mybir as mybir
from concourse.bass_utils import run_bass_kernel_spmd
from concourse.library_config import mlp

NUM_TABLES = 8
NUM_EMBEDDINGS = 100000
EMBED_DIM = 128
IDS_PER_FEATURE = 200000

P = 128
ROWS_PAD = 200704            # 1568 * 128 (704 pad rows, idx 0)
WSPAN = 32768                # out-tensor rows per window
WREAL = 32256                # real rows per window; rest is garbage pad
GARBAGE = WSPAN - 1          # in-window scatter target for pad slots
NWIN = 7                     # 7 * 32256 = 225792 >= 200704
OUT_ROWS = NWIN * WSPAN      # 229376
CH = 25000                   # table chunk rows (int16 gather range)
NCHUNK = 4
GCAP = 1024                  # rows per dma_gather instruction
UNIT = 3072                  # rows per unit = 3 gathers + 1 scatter
# static bucket caps (rows): full windows Bin(32256,1/4)=8064+-78 ->
# 9216 (+15sig); last window (7168 rows incl 704 idx-0 pads in chunk 0:
# ~2320 max expected) -> 3072.
CAP_FULL = 9216
CAP_LAST = 3072
NBUF = 6                     # slot-buffer pipeline depth

BUCKETS = []                 # (window, chunk, cap, idx_col_offset)
_off = 0
for _w in range(NWIN):
    cap = CAP_FULL if _w < NWIN - 1 else CAP_LAST
    for _c in range(NCHUNK):
        BUCKETS.append((_w, _c, cap, _off))
        _off += cap // 16
IDX_COLS = _off
# uniform pipeline units of UNIT rows
UNITS = []                   # (window, chunk, idx_col_offset_of_unit)
for _w, _c, _cap, _boff in BUCKETS:
    for _k in range(_cap // UNIT):
        UNITS.append((_w, _c, _boff + _k * (UNIT // 16)))


def build_nc():
    """Per-core Bass program (SPMD: same static program on all cores)."""
    nc = bacc.Bacc("TRN2", num_swdge_queues=4)
    gidx = nc.dram_tensor("gidx", [P, IDX_COLS], mybir.dt.int16,
                          kind="ExternalInput")
    sidx = nc.dram_tensor("sidx", [P, IDX_COLS], mybir.dt.int16,
                          kind="ExternalInput")
    tab = nc.dram_tensor("tab", [NUM_EMBEDDINGS, EMBED_DIM],
                         mybir.dt.float32, kind="ExternalInput")
    out = nc.dram_tensor("out", [OUT_ROWS, EMBED_DIM], mybir.dt.float32,
                         kind="ExternalOutput")

    nu = len(UNITS)
    with ExitStack() as es:
        block = es.enter_context(nc.Block())
        i_sem = es.enter_context(nc.semaphore("i_sem"))
        g_sems = [es.enter_context(nc.semaphore(f"g_sem{r}"))
                  for r in range(NBUF)]
        s_sems = [es.enter_context(nc.semaphore(f"s_sem{r}"))
                  for r in range(NBUF)]
        gidx_sb = es.enter_context(
            nc.sbuf_tensor("gidx_sb", [P, IDX_COLS], mybir.dt.int16))
        sidx_sb = es.enter_context(
            nc.sbuf_tensor("sidx_sb", [P, IDX_COLS], mybir.dt.int16))
        slots = [
            es.enter_context(
                nc.sbuf_tensor(f"slots{r}", [P, UNIT // P, EMBED_DIM],
                               mybir.dt.float32))
            for r in range(NBUF)
        ]

        g_done = [0] * NBUF   # gathers issued per buffer (build-time count)
        s_done = [0] * NBUF

        def gathers(gp, u):
            w, c, off = UNITS[u]
            r = u % NBUF
            for k in range(UNIT // GCAP):
                gp.dma_gather(
                    out_ap=slots[r][:, k * (GCAP // P):(k + 1) * (GCAP // P), :],
                    in_ap=tab[c * CH:(c + 1) * CH, :],
                    idxs_ap=gidx_sb[:, off + k * (GCAP // 16):
                                    off + (k + 1) * (GCAP // 16)],
                    num_idxs=GCAP,
                    num_idxs_reg=GCAP,
                    elem_size=EMBED_DIM,
                    single_packet=False,
                    queue_num=u % 4,
                ).then_inc(g_sems[r], 16)
            g_done[r] += UNIT // GCAP

        def scatter(gp, u):
            w, c, off = UNITS[u]
            r = u % NBUF
            gp.dma_scatter_add(
                out_ap=out[w * WSPAN:(w + 1) * WSPAN, :],
                in_ap=slots[r][:, :, :],
                idxs_ap=sidx_sb[:, off:off + UNIT // 16],
                num_idxs=UNIT,
                num_idxs_reg=UNIT,
                elem_size=EMBED_DIM,
                single_packet=False,
                queue_num=u % 4,
            ).then_inc(s_sems[r], 16)
            s_done[r] += 1

        @block.gpsimd
        def _(gp):
            gp.load_library(mlp)
            gp.dma_start(out=gidx_sb[:, :], in_=gidx[:, :]).then_inc(i_sem, 16)
            gp.dma_start(out=sidx_sb[:, :], in_=sidx[:, :]).then_inc(i_sem, 16)
            gp.wait_ge(i_sem, 32)
            for u in range(min(NBUF, nu)):
                gathers(gp, u)
            for u in range(nu):
                r = u % NBUF
                gp.wait_ge(g_sems[r], 16 * g_done[r])
                scatter(gp, u)
                if u + NBUF < nu:
                    gp.wait_ge(s_sems[r], 16 * s_done[r])
                    gathers(gp, u + NBUF)
            for r in range(NBUF):
                if s_done[r]:
                    gp.wait_ge(s_sems[r], 16 * s_done[r])
    nc.compile()
    return nc


_NC_CACHE = {}


def _get_nc():
    if "nc" not in _NC_CACHE:
        _NC_CACHE["nc"] = build_nc()
    return _NC_CACHE["nc"]


def _wrap16(v):
    """[N] int16 -> [128, N/16] wrapped in 16 partitions, replicated x8."""
    w = v.reshape(-1, 16).T
    return np.ascontiguousarray(np.tile(w, (8, 1)))


def _build_core_idx(idx_i32):
    """Bucket-sort one core's padded indices into gidx/sidx tensors."""
    gi_all = np.empty(IDX_COLS * 16, dtype=np.int16)
    si_all = np.empty(IDX_COLS * 16, dtype=np.int16)
    pos = np.arange(ROWS_PAD, dtype=np.int32)
    w_of = pos // WREAL
    r_of = pos % WREAL
    chunk_of = idx_i32 // CH
    for w, c, cap, off in BUCKETS:
        sel = pos[(w_of == w) & (chunk_of == c)]
        n = len(sel)
        assert n <= cap, f"bucket ({w},{c}) overflow: {n} > {cap}"
        g = np.zeros(cap, dtype=np.int16)            # pad -> chunk row 0
        s = np.full(cap, GARBAGE, dtype=np.int16)    # pad -> garbage row
        g[:n] = (idx_i32[sel] - c * CH).astype(np.int16)
        s[:n] = r_of[sel].astype(np.int16)
        o16 = off * 16
        gi_all[o16:o16 + cap] = g
        si_all[o16:o16 + cap] = s
    return {"gidx": _wrap16(gi_all), "sidx": _wrap16(si_all)}


def run(values: np.ndarray, weights: np.ndarray, trace: bool = False, **kw):
    assert values.shape == (NUM_TABLES, IDS_PER_FEATURE)
    assert weights.shape == (NUM_TABLES, NUM_EMBEDDINGS, EMBED_DIM)

    nc = _get_nc()

    idx_pad = np.zeros((NUM_TABLES, ROWS_PAD), dtype=np.int32)
    idx_pad[:, :IDS_PER_FEATURE] = values.astype(np.int32)
    w = np.ascontiguousarray(weights, dtype=np.float32)
    in_maps = [
        {**_build_core_idx(idx_pad[c]), "tab": w[c]}
        for c in range(NUM_TABLES)
    ]
    res = run_bass_kernel_spmd(nc, in_maps, core_ids=list(range(NUM_TABLES)),
                               trace=trace, **kw)
    outs = [
        r["out"].reshape(NWIN, WSPAN, EMBED_DIM)[:, :WREAL]
        .reshape(-1, EMBED_DIM)[:IDS_PER_FEATURE]
        for r in res.results
    ]
    return np.concatenate(outs, axis=0), res


def kernel(values: np.ndarray, weights: np.ndarray) -> np.ndarray:
    return run(values, weights)[0]


# revision 10
# speedup vs baseline: 2.9316x; 2.5126x over previous
"""GroupedEmbedding lookup on 8 Trainium2 NeuronCores.

Problem: 8 tables [100000, 128] f32, 8 index vectors [200000] int64.
Output: per-table gather concatenated -> [1600000, 128] f32.

Sharding: table-parallel. Core c holds table c and its 200000 indices;
it gathers locally. No collectives. Host concatenates the 8 slices.

Per-core kernel (v4, hybrid): two independent data-dependent movers
run concurrently, each saturating a different hardware resource:

  Path A (68% of rows, output tiles 0..1063): indirect1d DMA gathers
    (128 rows / ~1.4us gpsimd instruction; this cost is a hard floor:
    994ns ucode + ~310ns sequencer dispatch, measured) into SBUF
    group buffers, drained by cheap HWDGE stores on the sync engine.
    Binds: gpsimd engine (~11ns/row). DMA engines: ~3.6ns/row.

  Path B (32% of rows, the last 64512 = 2 windows of 32256): Q7 "ant"
    ucode ops (library mlp): dma_gather (1024 int16 idxs/instr from a
    25000-row table chunk) + dma_scatter_add (3072 idxs/instr into a
    32768-row output window; PJRT zero-donates outputs so += works).
    Rows are host-sorted into (window x chunk) buckets padded to
    static caps; pad gathers read chunk row 0, pad scatters hit the
    window's reserved garbage row 32767 (windows hold 32256 real
    rows), so pads never race real data and the program is
    input-independent. Binds: DMA engines (~150ns/packet, two packets
    per row = ~18.5ns/row over the 16-engine pool). gpsimd: ~3ns/row.

  B's ant ops use SWDGE queues 1-3 (num_swdge_queues=4); A's
  indirect1d stays on queue 0. B units (3 gathers + 1 scatter, 3072
  rows) are interleaved between A gathers so both DMA streams flow
  continuously. Resources balance at ~1.7ms, vs 2.25ms for pure
  indirect1d (engine-bound) and 6.5ms for pure ant (packet-bound).
"""

import os
import sys

for _p in ("/root/.axon_site", "/root/.axon_site/_ro/trn_rl_repo",
           "/root/.axon_site/_ro/pypackages", "/opt/trn_rl_repo"):
    if os.path.isdir(_p) and _p not in sys.path:
        sys.path.append(_p)

from contextlib import ExitStack

import numpy as np

import concourse.bacc as bacc
import concourse.bass as bass
import concourse.mybir as mybir
from concourse.bass_utils import run_bass_kernel_spmd
from concourse.library_config import mlp

NUM_TABLES = 8
NUM_EMBEDDINGS = 100000
EMBED_DIM = 128
IDS_PER_FEATURE = 200000

P = 128
ROWS_PAD = 200704            # 1568 * 128 (704 pad rows at the end, idx 0)

# --- path A (indirect1d) ---
GROUP = 56                   # tiles per store group
NGROUPS = 19
TA = GROUP * NGROUPS         # 1064 tiles
ROWS_A = TA * P              # 136192

# --- path B (ant bucket-bipartite) ---
ROWS_B = ROWS_PAD - ROWS_A   # 64512 = 2 * 32256
WSPAN = 32768                # out-tensor rows per window
WREAL = 32256                # real rows per window
GARBAGE = WSPAN - 1          # reserved garbage row (pad scatter target)
NWIN = ROWS_B // WREAL       # 2
OUTB_ROWS = NWIN * WSPAN
CH = 25000                   # table chunk rows (int16 gather range)
NCHUNK = 4
GCAP = 1024                  # rows per dma_gather (hard ucode cap)
UNIT = 3072                  # rows per unit = 3 gathers + 1 scatter (cap 3072)
CAP_B = 9216                 # bucket cap: Bin(32256,1/4)=8064+-78 (+704 pads
                             # in the last window chunk-0 bucket) -> +6..15sig
NBUF = 4                     # B slot-buffer pipeline depth

BUCKETS = []                 # (window, chunk, idx_col_offset)
_off = 0
for _w in range(NWIN):
    for _c in range(NCHUNK):
        BUCKETS.append((_w, _c, _off))
        _off += CAP_B // 16
IDX_COLS = _off
UNITS = []                   # (window, chunk, idx_col_offset_of_unit)
for _w, _c, _boff in BUCKETS:
    for _k in range(CAP_B // UNIT):
        UNITS.append((_w, _c, _boff + _k * (UNIT // 16)))
NU = len(UNITS)              # 24


def build_nc():
    """Per-core Bass program (SPMD: same static program on all cores)."""
    nc = bacc.Bacc("TRN2", num_swdge_queues=4)
    idxa = nc.dram_tensor("idxa", [P, TA], mybir.dt.int32,
                          kind="ExternalInput")
    gidx = nc.dram_tensor("gidx", [P, IDX_COLS], mybir.dt.int16,
                          kind="ExternalInput")
    sidx = nc.dram_tensor("sidx", [P, IDX_COLS], mybir.dt.int16,
                          kind="ExternalInput")
    tab = nc.dram_tensor("tab", [NUM_EMBEDDINGS, EMBED_DIM],
                         mybir.dt.float32, kind="ExternalInput")
    outa = nc.dram_tensor("outa", [TA, P, EMBED_DIM], mybir.dt.float32,
                          kind="ExternalOutput")
    outb = nc.dram_tensor("outb", [OUTB_ROWS, EMBED_DIM], mybir.dt.float32,
                          kind="ExternalOutput")

    with ExitStack() as es:
        block = es.enter_context(nc.Block())
        i_sem = es.enter_context(nc.semaphore("i_sem"))
        ga_sems = [es.enter_context(nc.semaphore(f"ga_sem{b}")) for b in (0, 1)]
        wa_sems = [es.enter_context(nc.semaphore(f"wa_sem{b}")) for b in (0, 1)]
        gb_sems = [es.enter_context(nc.semaphore(f"gb_sem{r}"))
                   for r in range(NBUF)]
        sb_sems = [es.enter_context(nc.semaphore(f"sb_sem{r}"))
                   for r in range(NBUF)]
        idxa_sb = es.enter_context(
            nc.sbuf_tensor("idxa_sb", [P, TA], mybir.dt.int32))
        gidx_sb = es.enter_context(
            nc.sbuf_tensor("gidx_sb", [P, IDX_COLS], mybir.dt.int16))
        sidx_sb = es.enter_context(
            nc.sbuf_tensor("sidx_sb", [P, IDX_COLS], mybir.dt.int16))
        gbuf = es.enter_context(
            nc.sbuf_tensor("gbuf", [P, 2 * GROUP * EMBED_DIM],
                           mybir.dt.float32))
        slots = [
            es.enter_context(
                nc.sbuf_tensor(f"slots{r}", [P, UNIT // P, EMBED_DIM],
                               mybir.dt.float32))
            for r in range(NBUF)
        ]

        gb_done = [0] * NBUF
        sb_done = [0] * NBUF

        def b_gathers(gp, u):
            w, c, off = UNITS[u]
            r = u % NBUF
            if u >= NBUF:
                gp.wait_ge(sb_sems[r], 16 * sb_done[r])
            for k in range(UNIT // GCAP):
                gp.dma_gather(
                    out_ap=slots[r][:, k * (GCAP // P):(k + 1) * (GCAP // P), :],
                    in_ap=tab[c * CH:(c + 1) * CH, :],
                    idxs_ap=gidx_sb[:, off + k * (GCAP // 16):
                                    off + (k + 1) * (GCAP // 16)],
                    num_idxs=GCAP,
                    num_idxs_reg=GCAP,
                    elem_size=EMBED_DIM,
                    single_packet=False,
                    queue_num=1 + u % 3,
                ).then_inc(gb_sems[r], 16)
            gb_done[r] += UNIT // GCAP

        def b_scatter(gp, u):
            w, c, off = UNITS[u]
            r = u % NBUF
            gp.wait_ge(gb_sems[r], 16 * gb_done[r])
            gp.dma_scatter_add(
                out_ap=outb[w * WSPAN:(w + 1) * WSPAN, :],
                in_ap=slots[r][:, :, :],
                idxs_ap=sidx_sb[:, off:off + UNIT // 16],
                num_idxs=UNIT,
                num_idxs_reg=UNIT,
                elem_size=EMBED_DIM,
                single_packet=False,
                queue_num=1 + u % 3,
            ).then_inc(sb_sems[r], 16)
            sb_done[r] += 1

        @block.gpsimd
        def _(gp):
            gp.load_library(mlp)
            gp.dma_start(out=idxa_sb[:, :], in_=idxa[:, :]).then_inc(i_sem, 16)
            gp.dma_start(out=gidx_sb[:, :], in_=gidx[:, :]).then_inc(i_sem, 16)
            gp.dma_start(out=sidx_sb[:, :], in_=sidx[:, :]).then_inc(i_sem, 16)
            gp.wait_ge(i_sem, 48)
            # B lag-pipeline: at each checkpoint issue gathers(u) then
            # scatter(u-1), so a unit's gather DMAs drain while ~28 A
            # gathers issue before its scatter needs them.
            b_next = [0]

            def b_step():
                u = b_next[0]
                if u < NU:
                    b_gathers(gp, u)
                    if u >= 1:
                        b_scatter(gp, u - 1)
                    b_next[0] = u + 1

            for k in range(NGROUPS):
                b = k % 2
                if k >= 2:
                    gp.wait_ge(wa_sems[b], 16 * (k // 2))
                for j in range(GROUP):
                    t = k * GROUP + j
                    o = (b * GROUP + j) * EMBED_DIM
                    gp.indirect_dma_start(
                        out=gbuf[:, o:o + EMBED_DIM],
                        out_offset=None,
                        in_=tab[:, :],
                        in_offset=bass.IndirectOffsetOnAxis(
                            ap=idxa_sb[:, t:t + 1], axis=0),
                    ).then_inc(ga_sems[b], 16)
                    if j % 28 == 27:
                        # 2 B units per A group ~= 38 checkpoints for 24
                        # units; b_step no-ops once all units issued
                        b_step()
            while b_next[0] < NU:
                b_step()
            if NU >= 1:
                b_scatter(gp, NU - 1)
            for r in range(NBUF):
                if sb_done[r]:
                    gp.wait_ge(sb_sems[r], 16 * sb_done[r])

        @block.sync
        def _(sy):
            for k in range(NGROUPS):
                b = k % 2
                sy.wait_ge(ga_sems[b], 16 * GROUP * (k // 2 + 1))
                sy.dma_start(
                    out=outa[k * GROUP:(k + 1) * GROUP].transpose([1, 0, 2]),
                    in_=gbuf[:, b * GROUP * EMBED_DIM:
                             (b + 1) * GROUP * EMBED_DIM],
                ).then_inc(wa_sems[b], 16)
    nc.compile()
    return nc


_NC_CACHE = {}


def _get_nc():
    if "nc" not in _NC_CACHE:
        _NC_CACHE["nc"] = build_nc()
    return _NC_CACHE["nc"]


def _wrap16(v):
    """[N] int16 -> [128, N/16] wrapped in 16 partitions, replicated x8."""
    w = v.reshape(-1, 16).T
    return np.ascontiguousarray(np.tile(w, (8, 1)))


def _build_core_inputs(idx_i32):
    """Host-side index prep for one core: A transpose + B bucket sort."""
    # A: idxa[p][t] = index of output row t*128+p
    idxa = np.ascontiguousarray(
        idx_i32[:ROWS_A].reshape(TA, P).transpose(1, 0))
    # B: bucket-sort rows ROWS_A.. into (window, chunk) buckets
    gi_all = np.empty(IDX_COLS * 16, dtype=np.int16)
    si_all = np.empty(IDX_COLS * 16, dtype=np.int16)
    bpos = np.arange(ROWS_B, dtype=np.int32)
    bidx = idx_i32[ROWS_A:]
    w_of = bpos // WREAL
    r_of = bpos % WREAL
    chunk_of = bidx // CH
    for w, c, off in BUCKETS:
        sel = bpos[(w_of == w) & (chunk_of == c)]
        n = len(sel)
        assert n <= CAP_B, f"bucket ({w},{c}) overflow: {n} > {CAP_B}"
        g = np.zeros(CAP_B, dtype=np.int16)           # pad -> chunk row 0
        s = np.full(CAP_B, GARBAGE, dtype=np.int16)   # pad -> garbage row
        g[:n] = (bidx[sel] - c * CH).astype(np.int16)
        s[:n] = r_of[sel].astype(np.int16)
        o16 = off * 16
        gi_all[o16:o16 + CAP_B] = g
        si_all[o16:o16 + CAP_B] = s
    return {"idxa": idxa, "gidx": _wrap16(gi_all), "sidx": _wrap16(si_all)}


def run(values: np.ndarray, weights: np.ndarray, trace: bool = False, **kw):
    assert values.shape == (NUM_TABLES, IDS_PER_FEATURE)
    assert weights.shape == (NUM_TABLES, NUM_EMBEDDINGS, EMBED_DIM)

    nc = _get_nc()

    idx_pad = np.zeros((NUM_TABLES, ROWS_PAD), dtype=np.int32)
    idx_pad[:, :IDS_PER_FEATURE] = values.astype(np.int32)
    w = np.ascontiguousarray(weights, dtype=np.float32)
    in_maps = [
        {**_build_core_inputs(idx_pad[c]), "tab": w[c]}
        for c in range(NUM_TABLES)
    ]
    res = run_bass_kernel_spmd(nc, in_maps, core_ids=list(range(NUM_TABLES)),
                               trace=trace, **kw)
    outs = []
    for r in res.results:
        a = r["outa"].reshape(ROWS_A, EMBED_DIM)
        bfull = r["outb"].reshape(NWIN, WSPAN, EMBED_DIM)[:, :WREAL]
        bpart = bfull.reshape(-1, EMBED_DIM)[:IDS_PER_FEATURE - ROWS_A]
        outs.append(a)
        outs.append(bpart)
    return np.concatenate(outs, axis=0), res


def kernel(values: np.ndarray, weights: np.ndarray) -> np.ndarray:
    return run(values, weights)[0]
